# revision 24
# baseline (speedup 1.0000x reference)
"""CPFGNN Trainium2 kernel: 8-core SPMD Bass implementation (v5).

Math (exact simplifications of the reference):
  - lam = 2.0 always (w_off <= 0), so diag = 0 and prop(t) is a pure
    edge scatter-add: prop(t) = -D^-1/2 A^T D^-1/2 t, with A the
    (multi-)adjacency count matrix excluding self-loops and deg = out-degree.
  - The 11 CTC @ e_k matvecs batch into one CTC @ E (N x 11) pass.

v5 structure (vs v2/v3):
  - Chebyshev state is kept TRANSPOSED ([128 nodes, LT, C]): the per-hop
    combine + u-limb prep runs as a short chain of tiny 128-partition ops
    with per-partition dinv scale APs, instead of five 0.7us ops on
    10-partition tiles.  The AG-feeding chain after a strip's PSUM close
    is ~3us instead of ~14us.
  - Per-hop AllGather of the u limbs is split (tiles 0-3 after strip 0,
    tiles 4-9 after strip 2) and the next hop's DoubleRow burst is gated
    in halves (pairs 0-1 / pairs 2-4), so the collective hides under the
    tensor burst.
  - AG bounce -> SBUF regather is one strided DMA per AG.
  - h16 rows come from a cheap back-transpose and build a resident
    [110, NSH] history (no DRAM round trip).
  - Tail: CTC streams via 2-tile batched DMAs on three queues, E-AG chain
    overlaps the first CTC DMAs.
"""
import os
import sys

sys.path.insert(0, "/opt/trn_rl_repo")

import numpy as np
import ml_dtypes
from contextlib import ExitStack

N = 10000
E_EDGES = 320000
F_IN = 500
HID = 64
C = 10
RANK = 3
K = 10
NC = 8
NSH = N // NC              # 1250 nodes per core
LT = (NSH + 127) // 128    # 10 local node tiles (last partial: 98)
LLAST = NSH - 128 * (LT - 1)  # 98
GJT = NC * LT              # 80 global j-tiles
PAIRS = LT // 2            # 5 DoubleRow pairs per core-block
# (col0, width, first local node tile, #tiles)
STRIPS = [(0, 512, 0, 4), (512, 512, 4, 4), (1024, NSH - 1024, 8, 2)]
# tail CTC strips (psum matmul output is limited to one 512-f32 bank)
TSTRIPS = [(0, 512), (512, 512), (1024, NSH - 1024)]
UW = 32                    # fp8 u row: hi 0:10, mid 10:20 (pair stride must be 16-mult)
EW = 16                    # fp16 e row: 0:11
TA, TB = 4, LT - 4         # AG split: tiles 0:4 (pairs 0-1) / 4:10 (pairs 2-4)

NP_FP8 = ml_dtypes.float8_e4m3
NP_BF16 = ml_dtypes.bfloat16

_CACHE = {}


def _build_program():
    import concourse.bass as bass
    import concourse.tile as tile
    from concourse import bacc, mybir
    from concourse.masks import make_identity

    dt = mybir.dt
    FP8 = dt.float8e4
    FP16 = dt.float16
    F32 = dt.float32
    AF = mybir.ActivationFunctionType
    ALU = mybir.AluOpType
    DR = mybir.MatmulPerfMode.DoubleRow

    nc = bacc.Bacc("TRN2", target_bir_lowering=False, debug=False, num_devices=NC)

    # ---------------- DRAM I/O ----------------
    a_dram = nc.dram_tensor("a8", [N, NSH], FP8, kind="ExternalInput")
    featT_dram = nc.dram_tensor("featT", [F_IN, NSH], FP16, kind="ExternalInput")
    ctct_dram = nc.dram_tensor("ctct", [N, NSH], FP16, kind="ExternalInput")
    w1_dram = nc.dram_tensor("w1", [F_IN, HID], FP16, kind="ExternalInput")
    b1_dram = nc.dram_tensor("b1", [HID, 1], F32, kind="ExternalInput")
    w2_dram = nc.dram_tensor("w2", [HID, C], FP16, kind="ExternalInput")
    b2_dram = nc.dram_tensor("b2", [C, 1], F32, kind="ExternalInput")
    wp_dram = nc.dram_tensor("wp", [C, (K + 1) * RANK], FP16, kind="ExternalInput")
    bp_dram = nc.dram_tensor("bp", [RANK, K + 1], F32, kind="ExternalInput")
    gam_dram = nc.dram_tensor("gam", [RANK, K + 1], FP16, kind="ExternalInput")
    # transposed per-node dinv scales: [128, 3*LT], rows r*LT+t:
    # r=0: +dinv, r=1: -dinv, r=2: -2*dinv  (this core's node range)
    dinvt_dram = nc.dram_tensor("dinvt", [128, 3 * LT], F32, kind="ExternalInput")
    sel11_dram = nc.dram_tensor("sel11", [K + 1, (K + 1) * C], FP16, kind="ExternalInput")
    sel11t_dram = nc.dram_tensor("sel11t", [(K + 1) * C, C], FP16, kind="ExternalInput")
    out_dram = nc.dram_tensor("out", [NSH, C], F32, kind="ExternalOutput")
    DEBUG = bool(os.environ.get("GNN_DEBUG"))
    if DEBUG:
        dump_e = nc.dram_tensor("dump_e", [K + 1, NSH], F32, kind="ExternalOutput")
        dump_eta = nc.dram_tensor("dump_eta", [K + 1, NSH], F32, kind="ExternalOutput")
        dump_hid = nc.dram_tensor("dump_hid", [C, NSH], F32, kind="ExternalOutput")

    RG = [list(range(NC))]

    with ExitStack() as ctx:
        tc = ctx.enter_context(tile.TileContext(nc))
        const = ctx.enter_context(tc.tile_pool(name="const", bufs=1))
        small = ctx.enter_context(tc.tile_pool(name="small", bufs=3))
        dram = ctx.enter_context(tc.tile_pool(name="dram", bufs=2, space="DRAM"))

        # Warm-up collective: triggers the one-time CC-stream barrier
        # (~51us) as early as possible so it overlaps the MLP + A load.
        wu_in = dram.tile([128, 8], FP16, tag="wuin", name="wuin", bufs=1)
        wu_out = dram.tile([NC, 128, 8], FP16, tag="wuout", name="wuout",
                           addr_space="Shared", bufs=1)
        with tc.high_priority():
            nc.gpsimd.collective_compute(
                "AllGather", ALU.bypass, replica_groups=RG,
                ins=[wu_in[:]], outs=[wu_out[:]],
            )

        # ------------- resident tensors -------------
        A8 = const.tile([128, NC, LT, NSH], FP8, tag="A8")
        u_stat = const.tile([128, NC, LT, UW], FP8, tag="u_stat")
        u_loc8 = const.tile([128, LT, UW], FP8, tag="u_loc8")
        e_stat = const.tile([128, NC, LT, EW], FP16, tag="e_stat")
        e_loc = const.tile([128, LT, EW], FP16, tag="e_loc")
        h110r = const.tile([(K + 1) * C, NSH], FP16, tag="h110r")
        # transposed Chebyshev state ring
        stT = [const.tile([128, LT, C], F32, tag=f"stT{i}", name=f"stT{i}")
               for i in range(3)]
        dts = const.tile([128, 3 * LT], F32, tag="dts")
        nc.sync.dma_start(dts[:], dinvt_dram[:])
        x2s = const.tile([C, NSH], F32, tag="x2s")

        w1s = const.tile([128, 4, HID], FP16, tag="w1")
        nc.sync.dma_start(
            w1s[:, 0:3, :], w1_dram[0:384, :].rearrange("(t p) c -> p t c", p=128)
        )
        nc.sync.dma_start(w1s[0:F_IN - 384, 3, :], w1_dram[384:F_IN, :])
        b1s = const.tile([HID, 1], F32, tag="b1")
        nc.sync.dma_start(b1s[:], b1_dram[:])
        w2s = const.tile([HID, C], FP16, tag="w2")
        nc.sync.dma_start(w2s[:], w2_dram[:])
        b2s = const.tile([C, 1], F32, tag="b2")
        nc.sync.dma_start(b2s[:], b2_dram[:])
        wps = const.tile([C, (K + 1) * RANK], FP16, tag="wp")
        nc.sync.dma_start(wps[:], wp_dram[:])
        bps = const.tile([RANK, K + 1], F32, tag="bp")
        nc.sync.dma_start(bps[:], bp_dram[:])
        gams = const.tile([RANK, K + 1], FP16, tag="gam")
        nc.sync.dma_start(gams[:], gam_dram[:])
        sel11s = const.tile([K + 1, (K + 1) * C], FP16, tag="sel11")
        nc.sync.dma_start(sel11s[:], sel11_dram[:])
        sel11Ts = const.tile([(K + 1) * C, C], FP16, tag="sel11t")
        nc.sync.dma_start(sel11Ts[:], sel11t_dram[:])
        ident = const.tile([128, 128], F32, tag="ident")
        make_identity(nc, ident[:])

        eT = const.tile([K + 1, NSH], F32, tag="eT")

        # zero DoubleRow pad rows (tile LT-1 has only LLAST valid rows).
        # Engine APs must start at a 32-aligned partition, so zero from 96;
        # rows 96..97 are rewritten by the A DMA / per-hop limb writes.
        nc.vector.memset(A8[96:128, :, LT - 1, :], 0.0)
        nc.vector.memset(u_loc8[:], 0.0)
        nc.vector.memset(u_stat[96:128, :, LT - 1, :], 0.0)
        nc.vector.memset(e_loc[96:128, LT - 1, :], 0.0)

        # A load on the gpsimd queue (idle until the first collective).
        for cg in range(NC):
            r0 = cg * NSH
            nc.gpsimd.dma_start(
                A8[:, cg, 0:LT - 1, :],
                a_dram[r0:r0 + 128 * (LT - 1), :].rearrange("(t p) c -> p t c", p=128),
            )
            nc.gpsimd.dma_start(A8[0:LLAST, cg, LT - 1, :],
                                a_dram[r0 + 128 * (LT - 1):r0 + NSH, :])

        KT = [(0, 128), (128, 128), (256, 128), (384, F_IN - 384)]
        h16_of = {}
        with tc.tile_pool(name="tmp0", bufs=1) as tmp0, \
             tc.tile_pool(name="mlps", bufs=4) as mlps, \
             tc.tile_pool(name="psmlp", bufs=3, space="PSUM") as psmlp:
            x1T = tmp0.tile([HID, NSH], FP16, tag="x1T")
            # ---------------- MLP (ki-major so 4 stream bufs suffice) --------
            pss1 = [psmlp.tile([HID, 512], F32, space="PSUM", tag=f"psA{si}",
                               name=f"psA{si}", bufs=1) for si in range(3)]
            for ki, (k0, kw) in enumerate(KT):
                ft = mlps.tile([128, NSH], FP16, tag="mv", name=f"ft{ki}")
                nc.sync.dma_start(ft[0:kw, :], featT_dram[k0:k0 + kw, :])
                for si, (s0, sw, _, _) in enumerate(STRIPS):
                    nc.tensor.matmul(
                        pss1[si][:, 0:sw], w1s[0:kw, ki, :], ft[0:kw, s0:s0 + sw],
                        start=(ki == 0), stop=(ki == 3),
                    )
            for si, (s0, sw, _, _) in enumerate(STRIPS):
                nc.scalar.activation(x1T[:, s0:s0 + sw], pss1[si][:, 0:sw], AF.Relu,
                                     bias=b1s[:], scale=1.0)
            for si, (s0, sw, _, _) in enumerate(STRIPS):
                ps2 = psmlp.tile([C, 512], F32, space="PSUM", tag="ps2", name="psB")
                nc.tensor.matmul(ps2[:, 0:sw], w2s[:], x1T[:, s0:s0 + sw],
                                 start=True, stop=True)
                nc.scalar.activation(x2s[:, s0:s0 + sw], ps2[:, 0:sw], AF.Identity,
                                     bias=b2s[:], scale=1.0)
                h16 = small.tile([C, 512], FP16, tag="h16", name=f"h16_0_{si}")
                nc.scalar.activation(h16[:, 0:sw], x2s[:, s0:s0 + sw], AF.Copy)
                nc.sync.dma_start(h110r[0:C, s0:s0 + sw], h16[:, 0:sw])
                h16_of[si] = h16

        # -------- hop-phase pools: 3 strip psum banks + 2+2 aux banks --------
        with tc.tile_pool(name="psH", bufs=1, space="PSUM") as psH, \
             tc.tile_pool(name="psX", bufs=2, space="PSUM") as psX, \
             tc.tile_pool(name="psB2", bufs=2, space="PSUM") as psB2, \
             tc.tile_pool(name="psXE", bufs=1, space="PSUM") as psXE:

            def limb_tile(curT, t, tag):
                """u-limb extraction for node tile t from transposed state."""
                pw = 128 if t < LT - 1 else LLAST
                nc.scalar.activation(u_loc8[0:pw, t, 0:10], curT[0:pw, t, :],
                                     AF.Copy, scale=dts[0:pw, t:t + 1])
                r1 = small.tile([128, C], F32, tag="r1", name=f"r1_{tag}_{t}")
                nc.vector.scalar_tensor_tensor(
                    out=r1[0:pw, :], in0=curT[0:pw, t, :],
                    scalar=dts[0:pw, t:t + 1], in1=u_loc8[0:pw, t, 0:10],
                    op0=ALU.mult, op1=ALU.subtract)
                nc.vector.tensor_scalar_mul(u_loc8[0:pw, t, 10:20], r1[0:pw, :],
                                            64.0)

            def launch_ag(t0, nt, k, part):
                """Stage u_loc8 tiles [t0, t0+nt) to DRAM, AllGather, regather."""
                agi = dram.tile([128, nt * UW], FP8, tag=f"agi{part}",
                                name=f"agi{part}{k}")
                ago = dram.tile([NC, 128, nt * UW], FP8, tag=f"ago{part}",
                                name=f"ago{part}{k}", addr_space="Shared")
                nc.sync.dma_start(agi[:], u_loc8[:, t0:t0 + nt, :])
                nc.gpsimd.collective_compute(
                    "AllGather", ALU.bypass, replica_groups=RG,
                    ins=[agi[:]], outs=[ago[:]],
                )
                h = (nt // 2) * UW
                nc.gpsimd.dma_start(
                    out=u_stat[:, :, t0:t0 + nt // 2, :],
                    in_=ago[:, :, 0:h].rearrange("c p x -> p c x"),
                )
                nc.scalar.dma_start(
                    out=u_stat[:, :, t0 + nt // 2:t0 + nt, :],
                    in_=ago[:, :, h:nt * UW].rearrange("c p x -> p c x"),
                )

            def compute_eta(k, h16s):
                """eT[k] = tanh(Txk @ Wp[k] + bp[k]) @ (gamma[:,k]/3)."""
                htas = []
                for si, (s0, sw, _, _) in enumerate(STRIPS):
                    psh = psXE.tile([128, 512], F32, space="PSUM", tag="auxE",
                                   name=f"psh{k}_{si}")
                    nc.tensor.matmul(psh[0:RANK, 0:sw],
                                     wps[:, k * RANK:(k + 1) * RANK],
                                     h16s[si][:, 0:sw], start=True, stop=True)
                    hta = small.tile([RANK, 512], FP16, tag="hta",
                                     name=f"hta{k}_{si}")
                    nc.scalar.activation(hta[:, 0:sw], psh[0:RANK, 0:sw], AF.Tanh,
                                         bias=bps[:, k:k + 1], scale=1.0)
                    htas.append(hta)
                eRow = small.tile([1, NSH], F32, tag="eRow", name=f"eRow{k}", bufs=1)
                for si, (s0, sw, _, _) in enumerate(STRIPS):
                    pse2 = psXE.tile([128, 512], F32, space="PSUM", tag="auxE",
                                    name=f"pse2{k}_{si}")
                    nc.tensor.matmul(pse2[0:1, 0:sw], gams[:, k:k + 1],
                                     htas[si][:, 0:sw], start=True, stop=True)
                    nc.vector.tensor_copy(eRow[:, s0:s0 + sw], pse2[0:1, 0:sw])
                nc.sync.dma_start(eT[k:k + 1, :], eRow[:])

            # ---------------- prologue (round 0) ----------------
            for si, (s0, sw, t0, nt) in enumerate(STRIPS):
                for ti in range(nt):
                    t = t0 + ti
                    pw = 128 if t < LT - 1 else LLAST
                    psT = psX.tile([128, 512], F32, space="PSUM", tag="aux",
                                   name=f"psT_p_{t}")
                    nc.tensor.transpose(psT[0:pw, 0:C],
                                        x2s[:, s0 + ti * 128:s0 + ti * 128 + pw],
                                        ident[0:C, 0:C])
                    nc.scalar.activation(stT[0][0:pw, t, :], psT[0:pw, 0:C],
                                         AF.Copy)
                    limb_tile(stT[0], t, "p")
            launch_ag(0, LT, 0, "f")
            compute_eta(0, h16_of)

            # ---------------- hops ----------------
            cur_i, prev_i, free_i = 0, None, 1
            for k in range(1, K + 1):
                pss = []
                for si, (s0, sw, _, _) in enumerate(STRIPS):
                    ps = psH.tile([20, 512], F32, space="PSUM", tag=f"s{si}",
                                  name=f"hop{k}s{si}")
                    pss.append(ps)
                curT = stT[cur_i]
                nxtT = stT[free_i]
                h16s = {}

                def combine_strip(si):
                    """Evac + transposed Chebyshev combine + limbs + h16."""
                    s0, sw, t0, nt = STRIPS[si]
                    ps = pss[si]
                    ev = small.tile([20, 512], F32, tag="ev", name=f"ev_{k}_{si}",
                                    bufs=2)
                    nc.vector.tensor_copy(ev[:, 0:sw], ps[0:20, 0:sw])
                    psb = psB2.tile([C, 512], F32, space="PSUM", tag="auxB",
                                    name=f"psb_{k}_{si}")
                    for ti in range(nt):
                        t = t0 + ti
                        pw = 128 if t < LT - 1 else LLAST
                        psT = psX.tile([128, 512], F32, space="PSUM", tag="aux",
                                       name=f"psT_{k}_{t}")
                        nc.tensor.transpose(psT[0:pw, 0:20],
                                            ev[:, ti * 128:ti * 128 + pw],
                                            ident[0:20, 0:20])
                        cp = small.tile([128, C], F32, tag="cp",
                                        name=f"cp_{k}_{t}")
                        nc.scalar.activation(cp[0:pw, :], psT[0:pw, 10:20],
                                             AF.Copy)
                        a2T = small.tile([128, C], F32, tag="a2T",
                                         name=f"a2T_{k}_{t}")
                        nc.vector.scalar_tensor_tensor(
                            out=a2T[0:pw, :], in0=cp[0:pw, :], scalar=1.0 / 64.0,
                            in1=psT[0:pw, 0:10], op0=ALU.mult, op1=ALU.add)
                        if k == 1:
                            nc.scalar.activation(nxtT[0:pw, t, :], a2T[0:pw, :],
                                                 AF.Copy,
                                                 scale=dts[0:pw, LT + t:LT + t + 1])
                        else:
                            nc.vector.scalar_tensor_tensor(
                                out=nxtT[0:pw, t, :], in0=a2T[0:pw, :],
                                scalar=dts[0:pw, 2 * LT + t:2 * LT + t + 1],
                                in1=stT[prev_i][0:pw, t, :],
                                op0=ALU.mult, op1=ALU.subtract)
                        if k < K:
                            limb_tile(nxtT, t, f"h{k}")
                        # back-transpose for the [C, cols] h16 row
                        nc.tensor.transpose(psb[0:C, ti * 128:ti * 128 + pw],
                                            nxtT[0:pw, t, :], ident[0:pw, 0:pw])
                    h16 = small.tile([C, 512], FP16, tag="h16", name=f"h16_{k}_{si}")
                    nc.scalar.activation(h16[:, 0:sw], psb[0:C, 0:sw], AF.Copy)
                    nc.sync.dma_start(h110r[k * C:(k + 1) * C, s0:s0 + sw],
                                      h16[:, 0:sw])
                    h16s[si] = h16

                # part A: pairs 0-1 (u tiles 0-3), all strips
                for si, (s0, sw, _, _) in enumerate(STRIPS):
                    for cg in range(NC):
                        for i in range(0, 2):
                            nc.tensor.matmul(
                                pss[si][:, 0:sw],
                                u_stat[:, cg, 2 * i:2 * i + 2, 0:20],
                                A8[:, cg, 2 * i:2 * i + 2, s0:s0 + sw],
                                start=(cg == 0 and i == 0), stop=False,
                                perf_mode=DR,
                            )
                # part B per strip, combine+prep interleaved
                for si, (s0, sw, _, _) in enumerate(STRIPS):
                    for cg in range(NC):
                        for i in range(2, PAIRS):
                            nc.tensor.matmul(
                                pss[si][:, 0:sw],
                                u_stat[:, cg, 2 * i:2 * i + 2, 0:20],
                                A8[:, cg, 2 * i:2 * i + 2, s0:s0 + sw],
                                start=False,
                                stop=(cg == NC - 1 and i == PAIRS - 1),
                                perf_mode=DR,
                            )
                    combine_strip(si)
                    if k < K:
                        if si == 0:
                            launch_ag(0, 2, k, "a1")
                            launch_ag(2, 2, k, "a2")
                        elif si == 2:
                            launch_ag(TA, TB, k, "b")
                prev_i, cur_i = cur_i, free_i
                free_i = 3 - cur_i - prev_i
                compute_eta(k, h16s)

        # ---------------- tail: E allgather + CTC @ E ----------------
        with tc.tile_pool(name="psT2", bufs=2, space="PSUM") as psT2, \
             tc.tile_pool(name="ctcs", bufs=8) as ctcs, \
             tc.tile_pool(name="tailp", bufs=1) as tailp:
            etaS = tailp.tile([K + 1, NSH], FP16, tag="etaS")
            hidT = tailp.tile([C, NSH], F32, tag="hidT")
            # E phase on gpsimd/tensor queues; CTC tile DMAs start
            # streaming concurrently on sync/scalar.
            for t in range(LT):
                pw = 128 if t < LT - 1 else LLAST
                psE = psT2.tile([128, 512], F32, space="PSUM", tag="aux2",
                                name=f"psE{t}")
                nc.tensor.transpose(psE[0:pw, 0:K + 1],
                                    eT[:, t * 128:t * 128 + pw],
                                    ident[0:K + 1, 0:K + 1])
                nc.scalar.activation(e_loc[0:pw, t, 0:K + 1], psE[0:pw, 0:K + 1],
                                     AF.Copy)
            if DEBUG:
                nc.sync.dma_start(dump_e[:], eT[:])
            for part, (t0, nt) in (("a", (0, TA)), ("b", (TA, TB))):
                agei = dram.tile([128, nt * EW], FP16, tag=f"agei{part}",
                                 name=f"agei{part}")
                ageo = dram.tile([NC, 128, nt * EW], FP16, tag=f"ageo{part}",
                                 name=f"ageo{part}", addr_space="Shared")
                nc.gpsimd.dma_start(agei[:], e_loc[:, t0:t0 + nt, :])
                nc.gpsimd.collective_compute(
                    "AllGather", ALU.bypass, replica_groups=RG,
                    ins=[agei[:]], outs=[ageo[:]],
                )
                nc.gpsimd.dma_start(out=e_stat[:, :, t0:t0 + nt, :],
                                    in_=ageo[:].rearrange("c p x -> p c x"))

            # ---------------- CTC @ E ----------------
            NPAIR = GJT // 2
            with tc.tile_pool(name="psC", bufs=1, space="PSUM") as psC:
                pcs = [psC.tile([K + 1, tw], F32, space="PSUM", tag=f"c{si}",
                                name=f"ctc{si}") for si, (s0, tw) in enumerate(TSTRIPS)]
                for jp in range(NPAIR):  # pair of j-tiles per DMA
                    pi, cg = jp // NC, jp % NC
                    row0 = cg * NSH + pi * 256
                    cj = ctcs.tile([128, 2, NSH], FP16, tag="cj", name=f"cj{jp}")
                    # sync/scalar take the early pairs; gpsimd (busy with the
                    # E chain first) takes the last ragged group.
                    if pi >= PAIRS - 2:
                        qeng = nc.gpsimd
                    else:
                        qeng = (nc.sync, nc.scalar)[jp % 2]
                    if pi < PAIRS - 1:
                        qeng.dma_start(
                            cj[:, :, :],
                            ctct_dram[row0:row0 + 256, :].rearrange(
                                "(t p) c -> p t c", p=128),
                        )
                    else:
                        qeng.dma_start(cj[:, 0, :], ctct_dram[row0:row0 + 128, :])
                        qeng.dma_start(cj[0:LLAST, 1, :],
                                       ctct_dram[row0 + 128:row0 + 128 + LLAST, :])
                    for tt in range(2):
                        t = 2 * pi + tt
                        kw = 128 if t < LT - 1 else LLAST
                        for si, (s0, sw) in enumerate(TSTRIPS):
                            nc.tensor.matmul(
                                pcs[si][:, 0:sw], e_stat[0:kw, cg, t, 0:K + 1],
                                cj[0:kw, tt, s0:s0 + sw],
                                start=(jp == 0 and tt == 0),
                                stop=(jp == NPAIR - 1 and tt == 1),
                            )
                for si, (s0, sw) in enumerate(TSTRIPS):
                    nc.vector.tensor_copy(etaS[:, s0:s0 + sw], pcs[si][:, 0:sw])
                if DEBUG:
                    etaSf = tailp.tile([K + 1, NSH], F32, tag="etaSf")
                    for si, (s0, sw) in enumerate(TSTRIPS):
                        nc.scalar.activation(etaSf[:, s0:s0 + sw],
                                             pcs[si][:, 0:sw], AF.Copy)
                    nc.sync.dma_start(dump_eta[:], etaSf[:])

            # ---------------- hidden = sum_k Txk * eta_k ----------------
            #   psb[110, s] = sel11^T @ etaS      (row k broadcast to 10 rows)
            #   prod = h110r * psb  (one 110-partition DVE op, fp16 out)
            #   hid[c, s] = sel11T^T @ prod       (sum over k groups)
            for si, (s0, sw, _, _) in enumerate(STRIPS):
                psb = psT2.tile([128, 512], F32, space="PSUM", tag="aux2",
                                name=f"psbh{si}")
                nc.tensor.matmul(psb[0:(K + 1) * C, 0:sw],
                                 sel11s[:, 0:(K + 1) * C],
                                 etaS[:, s0:s0 + sw], start=True, stop=True)
                prod = tailp.tile([(K + 1) * C, 512], FP16, tag="prod",
                                  name=f"prod{si}", bufs=2)
                nc.vector.tensor_tensor(out=prod[:, 0:sw],
                                        in0=h110r[:, s0:s0 + sw],
                                        in1=psb[0:(K + 1) * C, 0:sw],
                                        op=ALU.mult)
                ps10 = psT2.tile([128, 512], F32, space="PSUM", tag="aux2",
                                 name=f"ps10_{si}")
                nc.tensor.matmul(ps10[0:C, 0:sw], sel11Ts[:], prod[:, 0:sw],
                                 start=True, stop=True)
                nc.scalar.activation(hidT[:, s0:s0 + sw], ps10[0:C, 0:sw],
                                     AF.Copy)
            if DEBUG:
                nc.sync.dma_start(dump_hid[:], hidT[:])

            # ---------------- log_softmax + out ----------------
            pws = [128 if t < LT - 1 else LLAST for t in range(LT)]
            shs, sms, lss = [], [], []
            for t in range(LT):
                pw = pws[t]
                psS = psT2.tile([128, 512], F32, space="PSUM", tag="aux2",
                                name=f"psS{t}")
                nc.tensor.transpose(psS[0:pw, 0:C], hidT[:, t * 128:t * 128 + pw],
                                    ident[0:C, 0:C])
                h = small.tile([128, C], F32, tag="hrow", name=f"hrow{t}",
                               bufs=10)
                nc.vector.tensor_copy(h[0:pw, :], psS[0:pw, 0:C])
                mx = small.tile([128, 1], F32, tag="mx", name=f"mx{t}", bufs=10)
                nc.vector.tensor_reduce(mx[0:pw, :], h[0:pw, :],
                                        axis=mybir.AxisListType.X, op=ALU.max)
                sh = small.tile([128, C], F32, tag="sh", name=f"sh{t}", bufs=10)
                nc.vector.tensor_scalar_sub(sh[0:pw, :], h[0:pw, :], mx[0:pw, :])
                shs.append(sh)
            for t in range(LT):
                pw = pws[t]
                ex = small.tile([128, C], F32, tag="ex", name=f"ex{t}", bufs=2)
                sm = small.tile([128, 1], F32, tag="sm", name=f"sm{t}", bufs=10)
                nc.scalar.activation(ex[0:pw, :], shs[t][0:pw, :], AF.Exp,
                                     accum_out=sm[0:pw, :])
                sms.append(sm)
            for t in range(LT):
                pw = pws[t]
                ls = small.tile([128, 1], F32, tag="ls", name=f"ls{t}", bufs=10)
                nc.scalar.activation(ls[0:pw, :], sms[t][0:pw, :], AF.Ln)
                lss.append(ls)
            ob = tailp.tile([128, LT, C], F32, tag="ob")
            for t in range(LT):
                pw = pws[t]
                nc.vector.tensor_scalar_sub(ob[0:pw, t, :], shs[t][0:pw, :],
                                            lss[t][0:pw, :])
            nc.sync.dma_start(
                out_dram[0:128 * (LT - 1), :].rearrange("(t p) c -> p t c", p=128),
                ob[:, 0:LT - 1, :])
            nc.sync.dma_start(out_dram[128 * (LT - 1):NSH, :],
                              ob[0:LLAST, LT - 1, :])

    nc.compile()
    return nc


def _host_prep(feature, edges, CTC, W1, b1, W2, b2, gamma, Wp, bp):
    src = np.asarray(edges[0], dtype=np.int64)
    dst = np.asarray(edges[1], dtype=np.int64)
    nonself = src != dst
    s, d = src[nonself], dst[nonself]

    deg = np.bincount(s, minlength=N).astype(np.float64)
    dinv = np.where(deg > 0, 1.0 / np.sqrt(np.maximum(deg, 1e-30)), 0.0).astype(np.float32)

    counts = np.zeros((N, N), dtype=np.uint8)
    np.add.at(counts, (s, d), 1)
    lut = np.arange(256).astype(NP_FP8)
    a8 = lut[counts]          # [N, N] fp8, exact small ints

    feature = np.asarray(feature, dtype=np.float32)
    CTC = np.asarray(CTC, dtype=np.float32)

    sel11 = np.zeros((K + 1, (K + 1) * C), dtype=np.float16)
    for r in range(K + 1):
        sel11[r, r * C:(r + 1) * C] = 1.0
    sel11t = np.zeros(((K + 1) * C, C), dtype=np.float16)
    for r in range(K + 1):
        for c in range(C):
            sel11t[r * C + c, c] = 1.0

    in_maps = []
    for k in range(NC):
        r0, r1 = k * NSH, (k + 1) * NSH
        dloc = dinv[r0:r1]
        # transposed dinv scales [128, 3*LT]
        dinvt = np.zeros((128, 3 * LT), dtype=np.float32)
        for t in range(LT):
            pw = 128 if t < LT - 1 else LLAST
            col = dloc[t * 128:t * 128 + pw]
            dinvt[:pw, t] = col
            dinvt[:pw, LT + t] = -col
            dinvt[:pw, 2 * LT + t] = -2.0 * col
        in_maps.append({
            "a8": np.ascontiguousarray(a8[:, r0:r1]),
            "featT": np.ascontiguousarray(feature[r0:r1].T.astype(np.float16)),
            "ctct": np.ascontiguousarray(CTC[r0:r1].astype(np.float16).T),
            "w1": np.asarray(W1, dtype=np.float16),
            "b1": np.asarray(b1, dtype=np.float32).reshape(HID, 1),
            "w2": np.asarray(W2, dtype=np.float16),
            "b2": np.asarray(b2, dtype=np.float32).reshape(C, 1),
            "wp": np.ascontiguousarray(np.asarray(Wp, dtype=np.float32).transpose(1, 0, 2).reshape(C, (K + 1) * RANK)).astype(np.float16),
            "bp": np.ascontiguousarray(np.asarray(bp, dtype=np.float32).T),
            "gam": (np.asarray(gamma, dtype=np.float32) / RANK).astype(np.float16),
            "dinvt": dinvt,
            "sel11": sel11,
            "sel11t": sel11t,
        })
    return in_maps


def kernel(feature, edges, CTC, W1, b1, W2, b2, gamma, Wp, bp):
    from concourse.bass_utils import run_bass_kernel_spmd

    if "nc" not in _CACHE:
        _CACHE["nc"] = _build_program()
    nc = _CACHE["nc"]

    in_maps = _host_prep(feature, edges, CTC, W1, b1, W2, b2, gamma, Wp, bp)
    trace = bool(os.environ.get("GNN_TRACE"))
    res = run_bass_kernel_spmd(nc, in_maps, list(range(NC)), trace=trace)
    _CACHE["last_result"] = res
    out = np.concatenate([res.results[k]["out"] for k in range(NC)], axis=0)
    return out.astype(np.float32)


# revision 25
# speedup vs baseline: 1.0304x; 1.0304x over previous
"""CPFGNN Trainium2 kernel: 8-core SPMD Bass implementation (v5).

Math (exact simplifications of the reference):
  - lam = 2.0 always (w_off <= 0), so diag = 0 and prop(t) is a pure
    edge scatter-add: prop(t) = -D^-1/2 A^T D^-1/2 t, with A the
    (multi-)adjacency count matrix excluding self-loops and deg = out-degree.
  - The 11 CTC @ e_k matvecs batch into one CTC @ E (N x 11) pass.

v5 structure (vs v2/v3):
  - Chebyshev state is kept TRANSPOSED ([128 nodes, LT, C]): the per-hop
    combine + u-limb prep runs as a short chain of tiny 128-partition ops
    with per-partition dinv scale APs, instead of five 0.7us ops on
    10-partition tiles.  The AG-feeding chain after a strip's PSUM close
    is ~3us instead of ~14us.
  - Per-hop AllGather of the u limbs is split (tiles 0-3 after strip 0,
    tiles 4-9 after strip 2) and the next hop's DoubleRow burst is gated
    in halves (pairs 0-1 / pairs 2-4), so the collective hides under the
    tensor burst.
  - AG bounce -> SBUF regather is one strided DMA per AG.
  - h16 rows come from a cheap back-transpose and build a resident
    [110, NSH] history (no DRAM round trip).
  - Tail: CTC streams via 2-tile batched DMAs on three queues, E-AG chain
    overlaps the first CTC DMAs.
"""
import os
import sys

sys.path.insert(0, "/opt/trn_rl_repo")

import numpy as np
import ml_dtypes
from contextlib import ExitStack

N = 10000
E_EDGES = 320000
F_IN = 500
HID = 64
C = 10
RANK = 3
K = 10
NC = 8
NSH = N // NC              # 1250 nodes per core
LT = (NSH + 127) // 128    # 10 local node tiles (last partial: 98)
LLAST = NSH - 128 * (LT - 1)  # 98
GJT = NC * LT              # 80 global j-tiles
PAIRS = LT // 2            # 5 DoubleRow pairs per core-block
# (col0, width, first local node tile, #tiles)
STRIPS = [(0, 512, 0, 4), (512, 512, 4, 4), (1024, NSH - 1024, 8, 2)]
# tail CTC strips (psum matmul output is limited to one 512-f32 bank)
TSTRIPS = [(0, 512), (512, 512), (1024, NSH - 1024)]
UW = 32                    # fp8 u row: hi 0:10, mid 10:20 (pair stride must be 16-mult)
EW = 16                    # fp16 e row: 0:11
TA, TB = 4, LT - 4         # AG split: tiles 0:4 (pairs 0-1) / 4:10 (pairs 2-4)

NP_FP8 = ml_dtypes.float8_e4m3
NP_BF16 = ml_dtypes.bfloat16

_CACHE = {}


def _build_program():
    import concourse.bass as bass
    import concourse.tile as tile
    from concourse import bacc, mybir
    from concourse.masks import make_identity

    dt = mybir.dt
    FP8 = dt.float8e4
    FP16 = dt.float16
    F32 = dt.float32
    AF = mybir.ActivationFunctionType
    ALU = mybir.AluOpType
    DR = mybir.MatmulPerfMode.DoubleRow

    nc = bacc.Bacc("TRN2", target_bir_lowering=False, debug=False, num_devices=NC)

    # ---------------- DRAM I/O ----------------
    a_dram = nc.dram_tensor("a8", [N, NSH], FP8, kind="ExternalInput")
    featT_dram = nc.dram_tensor("featT", [F_IN, NSH], FP16, kind="ExternalInput")
    ctct_dram = nc.dram_tensor("ctct", [N, NSH], FP16, kind="ExternalInput")
    w1_dram = nc.dram_tensor("w1", [F_IN, HID], FP16, kind="ExternalInput")
    b1_dram = nc.dram_tensor("b1", [HID, 1], F32, kind="ExternalInput")
    w2_dram = nc.dram_tensor("w2", [HID, C], FP16, kind="ExternalInput")
    b2_dram = nc.dram_tensor("b2", [C, 1], F32, kind="ExternalInput")
    wp_dram = nc.dram_tensor("wp", [C, (K + 1) * RANK], FP16, kind="ExternalInput")
    bp_dram = nc.dram_tensor("bp", [RANK, K + 1], F32, kind="ExternalInput")
    gam_dram = nc.dram_tensor("gam", [RANK, K + 1], FP16, kind="ExternalInput")
    # transposed per-node dinv scales: [128, 3*LT], rows r*LT+t:
    # r=0: +dinv, r=1: -dinv, r=2: -2*dinv  (this core's node range)
    dinvt_dram = nc.dram_tensor("dinvt", [128, 3 * LT], F32, kind="ExternalInput")
    sel11_dram = nc.dram_tensor("sel11", [K + 1, (K + 1) * C], FP16, kind="ExternalInput")
    sel11t_dram = nc.dram_tensor("sel11t", [(K + 1) * C, C], FP16, kind="ExternalInput")
    out_dram = nc.dram_tensor("out", [NSH, C], F32, kind="ExternalOutput")
    DEBUG = bool(os.environ.get("GNN_DEBUG"))
    if DEBUG:
        dump_e = nc.dram_tensor("dump_e", [K + 1, NSH], F32, kind="ExternalOutput")
        dump_eta = nc.dram_tensor("dump_eta", [K + 1, NSH], F32, kind="ExternalOutput")
        dump_hid = nc.dram_tensor("dump_hid", [C, NSH], F32, kind="ExternalOutput")

    RG = [list(range(NC))]

    with ExitStack() as ctx:
        tc = ctx.enter_context(tile.TileContext(nc))
        const = ctx.enter_context(tc.tile_pool(name="const", bufs=1))
        small = ctx.enter_context(tc.tile_pool(name="small", bufs=3))
        dram = ctx.enter_context(tc.tile_pool(name="dram", bufs=2, space="DRAM"))

        # Warm-up collective: triggers the one-time CC-stream barrier
        # (~51us) as early as possible so it overlaps the MLP + A load.
        wu_in = dram.tile([128, 8], FP16, tag="wuin", name="wuin", bufs=1)
        wu_out = dram.tile([NC, 128, 8], FP16, tag="wuout", name="wuout",
                           addr_space="Shared", bufs=1)
        with tc.high_priority():
            nc.gpsimd.collective_compute(
                "AllGather", ALU.bypass, replica_groups=RG,
                ins=[wu_in[:]], outs=[wu_out[:]],
            )

        # ------------- resident tensors -------------
        A8 = const.tile([128, NC, LT, NSH], FP8, tag="A8")
        u_stat = const.tile([128, NC, LT, UW], FP8, tag="u_stat")
        u_loc8 = const.tile([128, LT, UW], FP8, tag="u_loc8")
        e_stat = const.tile([128, NC, LT, EW], FP16, tag="e_stat")
        e_loc = const.tile([128, LT, EW], FP16, tag="e_loc")
        h110r = const.tile([(K + 1) * C, NSH], FP16, tag="h110r")
        # transposed Chebyshev state ring
        stT = [const.tile([128, LT, C], F32, tag=f"stT{i}", name=f"stT{i}")
               for i in range(3)]
        dts = const.tile([128, 3 * LT], F32, tag="dts")
        nc.sync.dma_start(dts[:], dinvt_dram[:])
        x2s = const.tile([C, NSH], F32, tag="x2s")

        w1s = const.tile([128, 4, HID], FP16, tag="w1")
        nc.sync.dma_start(
            w1s[:, 0:3, :], w1_dram[0:384, :].rearrange("(t p) c -> p t c", p=128)
        )
        nc.sync.dma_start(w1s[0:F_IN - 384, 3, :], w1_dram[384:F_IN, :])
        b1s = const.tile([HID, 1], F32, tag="b1")
        nc.sync.dma_start(b1s[:], b1_dram[:])
        w2s = const.tile([HID, C], FP16, tag="w2")
        nc.sync.dma_start(w2s[:], w2_dram[:])
        b2s = const.tile([C, 1], F32, tag="b2")
        nc.sync.dma_start(b2s[:], b2_dram[:])
        wps = const.tile([C, (K + 1) * RANK], FP16, tag="wp")
        nc.sync.dma_start(wps[:], wp_dram[:])
        bps = const.tile([RANK, K + 1], F32, tag="bp")
        nc.sync.dma_start(bps[:], bp_dram[:])
        gams = const.tile([RANK, K + 1], FP16, tag="gam")
        nc.sync.dma_start(gams[:], gam_dram[:])
        sel11s = const.tile([K + 1, (K + 1) * C], FP16, tag="sel11")
        nc.sync.dma_start(sel11s[:], sel11_dram[:])
        sel11Ts = const.tile([(K + 1) * C, C], FP16, tag="sel11t")
        nc.sync.dma_start(sel11Ts[:], sel11t_dram[:])
        ident = const.tile([128, 128], F32, tag="ident")
        make_identity(nc, ident[:])

        eT = const.tile([K + 1, NSH], F32, tag="eT")

        # zero DoubleRow pad rows (tile LT-1 has only LLAST valid rows).
        # Engine APs must start at a 32-aligned partition, so zero from 96;
        # rows 96..97 are rewritten by the A DMA / per-hop limb writes.
        nc.vector.memset(A8[96:128, :, LT - 1, :], 0.0)
        nc.vector.memset(u_loc8[:], 0.0)
        nc.vector.memset(u_stat[96:128, :, LT - 1, :], 0.0)
        nc.vector.memset(e_loc[96:128, LT - 1, :], 0.0)

        # A load on the gpsimd queue (idle until the first collective).
        for cg in range(NC):
            r0 = cg * NSH
            nc.gpsimd.dma_start(
                A8[:, cg, 0:LT - 1, :],
                a_dram[r0:r0 + 128 * (LT - 1), :].rearrange("(t p) c -> p t c", p=128),
            )
            nc.gpsimd.dma_start(A8[0:LLAST, cg, LT - 1, :],
                                a_dram[r0 + 128 * (LT - 1):r0 + NSH, :])

        KT = [(0, 128), (128, 128), (256, 128), (384, F_IN - 384)]
        h16_of = {}
        with tc.tile_pool(name="tmp0", bufs=1) as tmp0, \
             tc.tile_pool(name="mlps", bufs=4) as mlps, \
             tc.tile_pool(name="psmlp", bufs=3, space="PSUM") as psmlp:
            x1T = tmp0.tile([HID, NSH], FP16, tag="x1T")
            # ---------------- MLP (ki-major so 4 stream bufs suffice) --------
            pss1 = [psmlp.tile([HID, 512], F32, space="PSUM", tag=f"psA{si}",
                               name=f"psA{si}", bufs=1) for si in range(3)]
            for ki, (k0, kw) in enumerate(KT):
                ft = mlps.tile([128, NSH], FP16, tag="mv", name=f"ft{ki}")
                nc.sync.dma_start(ft[0:kw, :], featT_dram[k0:k0 + kw, :])
                for si, (s0, sw, _, _) in enumerate(STRIPS):
                    nc.tensor.matmul(
                        pss1[si][:, 0:sw], w1s[0:kw, ki, :], ft[0:kw, s0:s0 + sw],
                        start=(ki == 0), stop=(ki == 3),
                    )
            for si, (s0, sw, _, _) in enumerate(STRIPS):
                nc.scalar.activation(x1T[:, s0:s0 + sw], pss1[si][:, 0:sw], AF.Relu,
                                     bias=b1s[:], scale=1.0)
            for si, (s0, sw, _, _) in enumerate(STRIPS):
                ps2 = psmlp.tile([C, 512], F32, space="PSUM", tag="ps2", name="psB")
                nc.tensor.matmul(ps2[:, 0:sw], w2s[:], x1T[:, s0:s0 + sw],
                                 start=True, stop=True)
                nc.scalar.activation(x2s[:, s0:s0 + sw], ps2[:, 0:sw], AF.Identity,
                                     bias=b2s[:], scale=1.0)
                h16 = small.tile([C, 512], FP16, tag="h16", name=f"h16_0_{si}")
                nc.scalar.activation(h16[:, 0:sw], x2s[:, s0:s0 + sw], AF.Copy)
                nc.sync.dma_start(h110r[0:C, s0:s0 + sw], h16[:, 0:sw])
                h16_of[si] = h16

        # -------- hop-phase pools: 3 strip psum banks + 2+2 aux banks --------
        with tc.tile_pool(name="psH", bufs=1, space="PSUM") as psH, \
             tc.tile_pool(name="psX", bufs=2, space="PSUM") as psX, \
             tc.tile_pool(name="psB2", bufs=2, space="PSUM") as psB2, \
             tc.tile_pool(name="psXE", bufs=1, space="PSUM") as psXE:

            def limb_tile(curT, t, tag):
                """u-limb extraction for node tile t from transposed state."""
                pw = 128 if t < LT - 1 else LLAST
                nc.scalar.activation(u_loc8[0:pw, t, 0:10], curT[0:pw, t, :],
                                     AF.Copy, scale=dts[0:pw, t:t + 1])
                r1 = small.tile([128, C], F32, tag="r1", name=f"r1_{tag}_{t}")
                nc.vector.scalar_tensor_tensor(
                    out=r1[0:pw, :], in0=curT[0:pw, t, :],
                    scalar=dts[0:pw, t:t + 1], in1=u_loc8[0:pw, t, 0:10],
                    op0=ALU.mult, op1=ALU.subtract)
                nc.vector.tensor_scalar_mul(u_loc8[0:pw, t, 10:20], r1[0:pw, :],
                                            64.0)

            def launch_ag(t0, nt, k, part):
                """Stage u_loc8 tiles [t0, t0+nt) to DRAM, AllGather, regather."""
                agi = dram.tile([128, nt * UW], FP8, tag=f"agi{part}",
                                name=f"agi{part}{k}")
                ago = dram.tile([NC, 128, nt * UW], FP8, tag=f"ago{part}",
                                name=f"ago{part}{k}", addr_space="Shared")
                nc.sync.dma_start(agi[:], u_loc8[:, t0:t0 + nt, :])
                nc.gpsimd.collective_compute(
                    "AllGather", ALU.bypass, replica_groups=RG,
                    ins=[agi[:]], outs=[ago[:]],
                )
                h = (nt // 2) * UW
                nc.gpsimd.dma_start(
                    out=u_stat[:, :, t0:t0 + nt // 2, :],
                    in_=ago[:, :, 0:h].rearrange("c p x -> p c x"),
                )
                nc.scalar.dma_start(
                    out=u_stat[:, :, t0 + nt // 2:t0 + nt, :],
                    in_=ago[:, :, h:nt * UW].rearrange("c p x -> p c x"),
                )

            def compute_eta(k, h16s):
                """eT[k] = tanh(Txk @ Wp[k] + bp[k]) @ (gamma[:,k]/3)."""
                htas = []
                for si, (s0, sw, _, _) in enumerate(STRIPS):
                    psh = psXE.tile([128, 512], F32, space="PSUM", tag="auxE",
                                   name=f"psh{k}_{si}")
                    nc.tensor.matmul(psh[0:RANK, 0:sw],
                                     wps[:, k * RANK:(k + 1) * RANK],
                                     h16s[si][:, 0:sw], start=True, stop=True)
                    hta = small.tile([RANK, 512], FP16, tag="hta",
                                     name=f"hta{k}_{si}")
                    nc.scalar.activation(hta[:, 0:sw], psh[0:RANK, 0:sw], AF.Tanh,
                                         bias=bps[:, k:k + 1], scale=1.0)
                    htas.append(hta)
                eRow = small.tile([1, NSH], F32, tag="eRow", name=f"eRow{k}", bufs=1)
                for si, (s0, sw, _, _) in enumerate(STRIPS):
                    pse2 = psXE.tile([128, 512], F32, space="PSUM", tag="auxE",
                                    name=f"pse2{k}_{si}")
                    nc.tensor.matmul(pse2[0:1, 0:sw], gams[:, k:k + 1],
                                     htas[si][:, 0:sw], start=True, stop=True)
                    nc.vector.tensor_copy(eRow[:, s0:s0 + sw], pse2[0:1, 0:sw])
                nc.sync.dma_start(eT[k:k + 1, :], eRow[:])

            # ---------------- prologue (round 0) ----------------
            for si, (s0, sw, t0, nt) in enumerate(STRIPS):
                for ti in range(nt):
                    t = t0 + ti
                    pw = 128 if t < LT - 1 else LLAST
                    psT = psX.tile([128, 512], F32, space="PSUM", tag="aux",
                                   name=f"psT_p_{t}")
                    nc.tensor.transpose(psT[0:pw, 0:C],
                                        x2s[:, s0 + ti * 128:s0 + ti * 128 + pw],
                                        ident[0:C, 0:C])
                    nc.scalar.activation(stT[0][0:pw, t, :], psT[0:pw, 0:C],
                                         AF.Copy)
                    limb_tile(stT[0], t, "p")
            launch_ag(0, LT, 0, "f")
            compute_eta(0, h16_of)

            # ---------------- hops ----------------
            cur_i, prev_i, free_i = 0, None, 1
            for k in range(1, K + 1):
                pss = []
                for si, (s0, sw, _, _) in enumerate(STRIPS):
                    ps = psH.tile([20, 512], F32, space="PSUM", tag=f"s{si}",
                                  name=f"hop{k}s{si}")
                    pss.append(ps)
                curT = stT[cur_i]
                nxtT = stT[free_i]
                h16s = {}

                def combine_strip(si):
                    """Evac + transposed Chebyshev combine + limbs + h16."""
                    s0, sw, t0, nt = STRIPS[si]
                    ps = pss[si]
                    ev = small.tile([20, 512], F32, tag="ev", name=f"ev_{k}_{si}",
                                    bufs=2)
                    nc.vector.tensor_copy(ev[:, 0:sw], ps[0:20, 0:sw])
                    psb = psB2.tile([C, 512], F32, space="PSUM", tag="auxB",
                                    name=f"psb_{k}_{si}")
                    for ti in range(nt):
                        t = t0 + ti
                        pw = 128 if t < LT - 1 else LLAST
                        psT = psX.tile([128, 512], F32, space="PSUM", tag="aux",
                                       name=f"psT_{k}_{t}")
                        nc.tensor.transpose(psT[0:pw, 0:20],
                                            ev[:, ti * 128:ti * 128 + pw],
                                            ident[0:20, 0:20])
                        cp = small.tile([128, C], F32, tag="cp",
                                        name=f"cp_{k}_{t}")
                        nc.scalar.activation(cp[0:pw, :], psT[0:pw, 10:20],
                                             AF.Copy)
                        a2T = small.tile([128, C], F32, tag="a2T",
                                         name=f"a2T_{k}_{t}")
                        nc.vector.scalar_tensor_tensor(
                            out=a2T[0:pw, :], in0=cp[0:pw, :], scalar=1.0 / 64.0,
                            in1=psT[0:pw, 0:10], op0=ALU.mult, op1=ALU.add)
                        if k == 1:
                            nc.scalar.activation(nxtT[0:pw, t, :], a2T[0:pw, :],
                                                 AF.Copy,
                                                 scale=dts[0:pw, LT + t:LT + t + 1])
                        else:
                            nc.vector.scalar_tensor_tensor(
                                out=nxtT[0:pw, t, :], in0=a2T[0:pw, :],
                                scalar=dts[0:pw, 2 * LT + t:2 * LT + t + 1],
                                in1=stT[prev_i][0:pw, t, :],
                                op0=ALU.mult, op1=ALU.subtract)
                        if k < K:
                            limb_tile(nxtT, t, f"h{k}")
                        # back-transpose for the [C, cols] h16 row
                        nc.tensor.transpose(psb[0:C, ti * 128:ti * 128 + pw],
                                            nxtT[0:pw, t, :], ident[0:pw, 0:pw])
                    h16 = small.tile([C, 512], FP16, tag="h16", name=f"h16_{k}_{si}")
                    nc.scalar.activation(h16[:, 0:sw], psb[0:C, 0:sw], AF.Copy)
                    nc.sync.dma_start(h110r[k * C:(k + 1) * C, s0:s0 + sw],
                                      h16[:, 0:sw])
                    h16s[si] = h16

                # part A: pairs 0-1 (u tiles 0-3), all strips
                for si, (s0, sw, _, _) in enumerate(STRIPS):
                    for cg in range(NC):
                        for i in range(0, 2):
                            nc.tensor.matmul(
                                pss[si][:, 0:sw],
                                u_stat[:, cg, 2 * i:2 * i + 2, 0:20],
                                A8[:, cg, 2 * i:2 * i + 2, s0:s0 + sw],
                                start=(cg == 0 and i == 0), stop=False,
                                perf_mode=DR,
                            )
                # part B per strip, combine+prep interleaved
                for si, (s0, sw, _, _) in enumerate(STRIPS):
                    for cg in range(NC):
                        for i in range(2, PAIRS):
                            nc.tensor.matmul(
                                pss[si][:, 0:sw],
                                u_stat[:, cg, 2 * i:2 * i + 2, 0:20],
                                A8[:, cg, 2 * i:2 * i + 2, s0:s0 + sw],
                                start=False,
                                stop=(cg == NC - 1 and i == PAIRS - 1),
                                perf_mode=DR,
                            )
                    combine_strip(si)
                    if k < K:
                        if si == 0:
                            launch_ag(0, TA, k, "a")
                        elif si == 2:
                            launch_ag(TA, TB, k, "b")
                prev_i, cur_i = cur_i, free_i
                free_i = 3 - cur_i - prev_i
                compute_eta(k, h16s)

        # ---------------- tail: E allgather + CTC @ E ----------------
        with tc.tile_pool(name="psT2", bufs=2, space="PSUM") as psT2, \
             tc.tile_pool(name="ctcs", bufs=10) as ctcs, \
             tc.tile_pool(name="tailp", bufs=1) as tailp:
            etaS = tailp.tile([K + 1, NSH], FP16, tag="etaS")
            hidT = tailp.tile([C, NSH], F32, tag="hidT")
            # E phase on gpsimd/tensor queues; CTC tile DMAs start
            # streaming concurrently on sync/scalar.
            for t in range(LT):
                pw = 128 if t < LT - 1 else LLAST
                psE = psT2.tile([128, 512], F32, space="PSUM", tag="aux2",
                                name=f"psE{t}")
                nc.tensor.transpose(psE[0:pw, 0:K + 1],
                                    eT[:, t * 128:t * 128 + pw],
                                    ident[0:K + 1, 0:K + 1])
                nc.scalar.activation(e_loc[0:pw, t, 0:K + 1], psE[0:pw, 0:K + 1],
                                     AF.Copy)
            if DEBUG:
                nc.sync.dma_start(dump_e[:], eT[:])
            for part, (t0, nt) in (("a", (0, TA)), ("b", (TA, TB))):
                agei = dram.tile([128, nt * EW], FP16, tag=f"agei{part}",
                                 name=f"agei{part}")
                ageo = dram.tile([NC, 128, nt * EW], FP16, tag=f"ageo{part}",
                                 name=f"ageo{part}", addr_space="Shared")
                nc.gpsimd.dma_start(agei[:], e_loc[:, t0:t0 + nt, :])
                nc.gpsimd.collective_compute(
                    "AllGather", ALU.bypass, replica_groups=RG,
                    ins=[agei[:]], outs=[ageo[:]],
                )
                nc.gpsimd.dma_start(out=e_stat[:, :, t0:t0 + nt, :],
                                    in_=ageo[:].rearrange("c p x -> p c x"))

            # ---------------- CTC @ E ----------------
            NPAIR = GJT // 2
            with tc.tile_pool(name="psC", bufs=1, space="PSUM") as psC:
                pcs = [psC.tile([K + 1, tw], F32, space="PSUM", tag=f"c{si}",
                                name=f"ctc{si}") for si, (s0, tw) in enumerate(TSTRIPS)]
                for jp in range(NPAIR):  # pair of j-tiles per DMA
                    pi, cg = jp // NC, jp % NC
                    row0 = cg * NSH + pi * 256
                    cj = ctcs.tile([128, 2, NSH], FP16, tag="cj", name=f"cj{jp}")
                    # sync/scalar take the early pairs; gpsimd (busy with the
                    # E chain first) takes the last ragged group.
                    if pi >= PAIRS - 2:
                        qeng = nc.gpsimd
                    else:
                        qeng = (nc.sync, nc.scalar)[jp % 2]
                    if pi < PAIRS - 1:
                        qeng.dma_start(
                            cj[:, :, :],
                            ctct_dram[row0:row0 + 256, :].rearrange(
                                "(t p) c -> p t c", p=128),
                        )
                    else:
                        qeng.dma_start(cj[:, 0, :], ctct_dram[row0:row0 + 128, :])
                        qeng.dma_start(cj[0:LLAST, 1, :],
                                       ctct_dram[row0 + 128:row0 + 128 + LLAST, :])
                    for tt in range(2):
                        t = 2 * pi + tt
                        kw = 128 if t < LT - 1 else LLAST
                        for si, (s0, sw) in enumerate(TSTRIPS):
                            nc.tensor.matmul(
                                pcs[si][:, 0:sw], e_stat[0:kw, cg, t, 0:K + 1],
                                cj[0:kw, tt, s0:s0 + sw],
                                start=(jp == 0 and tt == 0),
                                stop=(jp == NPAIR - 1 and tt == 1),
                            )
                for si, (s0, sw) in enumerate(TSTRIPS):
                    nc.vector.tensor_copy(etaS[:, s0:s0 + sw], pcs[si][:, 0:sw])
                if DEBUG:
                    etaSf = tailp.tile([K + 1, NSH], F32, tag="etaSf")
                    for si, (s0, sw) in enumerate(TSTRIPS):
                        nc.scalar.activation(etaSf[:, s0:s0 + sw],
                                             pcs[si][:, 0:sw], AF.Copy)
                    nc.sync.dma_start(dump_eta[:], etaSf[:])

            # ---------------- hidden = sum_k Txk * eta_k ----------------
            #   psb[110, s] = sel11^T @ etaS      (row k broadcast to 10 rows)
            #   prod = h110r * psb  (one 110-partition DVE op, fp16 out)
            #   hid[c, s] = sel11T^T @ prod       (sum over k groups)
            for si, (s0, sw, _, _) in enumerate(STRIPS):
                psb = psT2.tile([128, 512], F32, space="PSUM", tag="aux2",
                                name=f"psbh{si}")
                nc.tensor.matmul(psb[0:(K + 1) * C, 0:sw],
                                 sel11s[:, 0:(K + 1) * C],
                                 etaS[:, s0:s0 + sw], start=True, stop=True)
                prod = tailp.tile([(K + 1) * C, 512], FP16, tag="prod",
                                  name=f"prod{si}", bufs=2)
                nc.vector.tensor_tensor(out=prod[:, 0:sw],
                                        in0=h110r[:, s0:s0 + sw],
                                        in1=psb[0:(K + 1) * C, 0:sw],
                                        op=ALU.mult)
                ps10 = psT2.tile([128, 512], F32, space="PSUM", tag="aux2",
                                 name=f"ps10_{si}")
                nc.tensor.matmul(ps10[0:C, 0:sw], sel11Ts[:], prod[:, 0:sw],
                                 start=True, stop=True)
                nc.scalar.activation(hidT[:, s0:s0 + sw], ps10[0:C, 0:sw],
                                     AF.Copy)
            if DEBUG:
                nc.sync.dma_start(dump_hid[:], hidT[:])

            # ---------------- log_softmax + out ----------------
            pws = [128 if t < LT - 1 else LLAST for t in range(LT)]
            shs, sms, lss = [], [], []
            for t in range(LT):
                pw = pws[t]
                psS = psT2.tile([128, 512], F32, space="PSUM", tag="aux2",
                                name=f"psS{t}")
                nc.tensor.transpose(psS[0:pw, 0:C], hidT[:, t * 128:t * 128 + pw],
                                    ident[0:C, 0:C])
                h = small.tile([128, C], F32, tag="hrow", name=f"hrow{t}",
                               bufs=10)
                nc.vector.tensor_copy(h[0:pw, :], psS[0:pw, 0:C])
                mx = small.tile([128, 1], F32, tag="mx", name=f"mx{t}", bufs=10)
                nc.vector.tensor_reduce(mx[0:pw, :], h[0:pw, :],
                                        axis=mybir.AxisListType.X, op=ALU.max)
                sh = small.tile([128, C], F32, tag="sh", name=f"sh{t}", bufs=10)
                nc.vector.tensor_scalar_sub(sh[0:pw, :], h[0:pw, :], mx[0:pw, :])
                shs.append(sh)
            for t in range(LT):
                pw = pws[t]
                ex = small.tile([128, C], F32, tag="ex", name=f"ex{t}", bufs=2)
                sm = small.tile([128, 1], F32, tag="sm", name=f"sm{t}", bufs=10)
                nc.scalar.activation(ex[0:pw, :], shs[t][0:pw, :], AF.Exp,
                                     accum_out=sm[0:pw, :])
                sms.append(sm)
            for t in range(LT):
                pw = pws[t]
                ls = small.tile([128, 1], F32, tag="ls", name=f"ls{t}", bufs=10)
                nc.scalar.activation(ls[0:pw, :], sms[t][0:pw, :], AF.Ln)
                lss.append(ls)
            ob = tailp.tile([128, LT, C], F32, tag="ob")
            for t in range(LT):
                pw = pws[t]
                nc.vector.tensor_scalar_sub(ob[0:pw, t, :], shs[t][0:pw, :],
                                            lss[t][0:pw, :])
            nc.sync.dma_start(
                out_dram[0:128 * (LT - 1), :].rearrange("(t p) c -> p t c", p=128),
                ob[:, 0:LT - 1, :])
            nc.sync.dma_start(out_dram[128 * (LT - 1):NSH, :],
                              ob[0:LLAST, LT - 1, :])

    nc.compile()
    return nc


def _host_prep(feature, edges, CTC, W1, b1, W2, b2, gamma, Wp, bp):
    src = np.asarray(edges[0], dtype=np.int64)
    dst = np.asarray(edges[1], dtype=np.int64)
    nonself = src != dst
    s, d = src[nonself], dst[nonself]

    deg = np.bincount(s, minlength=N).astype(np.float64)
    dinv = np.where(deg > 0, 1.0 / np.sqrt(np.maximum(deg, 1e-30)), 0.0).astype(np.float32)

    counts = np.zeros((N, N), dtype=np.uint8)
    np.add.at(counts, (s, d), 1)
    lut = np.arange(256).astype(NP_FP8)
    a8 = lut[counts]          # [N, N] fp8, exact small ints

    feature = np.asarray(feature, dtype=np.float32)
    CTC = np.asarray(CTC, dtype=np.float32)

    sel11 = np.zeros((K + 1, (K + 1) * C), dtype=np.float16)
    for r in range(K + 1):
        sel11[r, r * C:(r + 1) * C] = 1.0
    sel11t = np.zeros(((K + 1) * C, C), dtype=np.float16)
    for r in range(K + 1):
        for c in range(C):
            sel11t[r * C + c, c] = 1.0

    in_maps = []
    for k in range(NC):
        r0, r1 = k * NSH, (k + 1) * NSH
        dloc = dinv[r0:r1]
        # transposed dinv scales [128, 3*LT]
        dinvt = np.zeros((128, 3 * LT), dtype=np.float32)
        for t in range(LT):
            pw = 128 if t < LT - 1 else LLAST
            col = dloc[t * 128:t * 128 + pw]
            dinvt[:pw, t] = col
            dinvt[:pw, LT + t] = -col
            dinvt[:pw, 2 * LT + t] = -2.0 * col
        in_maps.append({
            "a8": np.ascontiguousarray(a8[:, r0:r1]),
            "featT": np.ascontiguousarray(feature[r0:r1].T.astype(np.float16)),
            "ctct": np.ascontiguousarray(CTC[r0:r1].astype(np.float16).T),
            "w1": np.asarray(W1, dtype=np.float16),
            "b1": np.asarray(b1, dtype=np.float32).reshape(HID, 1),
            "w2": np.asarray(W2, dtype=np.float16),
            "b2": np.asarray(b2, dtype=np.float32).reshape(C, 1),
            "wp": np.ascontiguousarray(np.asarray(Wp, dtype=np.float32).transpose(1, 0, 2).reshape(C, (K + 1) * RANK)).astype(np.float16),
            "bp": np.ascontiguousarray(np.asarray(bp, dtype=np.float32).T),
            "gam": (np.asarray(gamma, dtype=np.float32) / RANK).astype(np.float16),
            "dinvt": dinvt,
            "sel11": sel11,
            "sel11t": sel11t,
        })
    return in_maps


def kernel(feature, edges, CTC, W1, b1, W2, b2, gamma, Wp, bp):
    from concourse.bass_utils import run_bass_kernel_spmd

    if "nc" not in _CACHE:
        _CACHE["nc"] = _build_program()
    nc = _CACHE["nc"]

    in_maps = _host_prep(feature, edges, CTC, W1, b1, W2, b2, gamma, Wp, bp)
    trace = bool(os.environ.get("GNN_TRACE"))
    res = run_bass_kernel_spmd(nc, in_maps, list(range(NC)), trace=trace)
    _CACHE["last_result"] = res
    out = np.concatenate([res.results[k]["out"] for k in range(NC)], axis=0)
    return out.astype(np.float32)
